# revision 1
# baseline (speedup 1.0000x reference)
"""Trainium2 Bass kernel for nn_ARAttention (axial-region top-k windowed attention).

Sharding: 8 cores = 4 batches x 2 branches (vertical / horizontal). The
horizontal branch is the vertical branch on the spatially-transposed image
(with a spatially-transposed LePE kernel), so all cores run one SPMD program.

Per-core layout conventions (all hardcoded for x[4,256,64,64]):
  - channel-major image  xc [256, 4096], pixel p = row*64 + col
  - windows = 32 contiguous 128-pixel blocks of the free axis
  - qkv channel-major [768, 4096] as SBUF [128, 6*4096] (6 channel chunks)
  - V token-major [128 pix-in-window, 32w * 256ch]
  - routing (window top-2) is computed exactly on host (mean is linear:
    mean(win) @ Wq + bq) and shipped as an int32 index tensor; on device the
    indices feed register-based dynamic slices of the matmul operands.
"""

import numpy as np
import ml_dtypes

DIM = 256
QK = 256
HEADS = 8
HD = 32  # head dim
N_B = 4
H = W = 64
P2 = 32  # windows per branch image
W2 = 128  # pixels per window
NPIX = 4096
SCALE = QK ** -0.5

# tap order: center first so its matmul (start=True) covers the full PSUM range
TAPS = [(0, 0), (-1, -1), (-1, 0), (-1, 1), (0, -1), (0, 1), (1, -1), (1, 0), (1, 1)]

_CACHE = {}


def _build_program():
    import concourse.bass as bass
    import concourse.bacc as bacc
    import concourse.mybir as mybir
    from concourse.bass import ds
    from concourse.tile import TileContext

    f32 = mybir.dt.float32
    bf16 = mybir.dt.bfloat16
    i32 = mybir.dt.int32
    AF = mybir.ActivationFunctionType

    nc = bacc.Bacc("TRN2", target_bir_lowering=False, debug=False)

    x_d = nc.declare_dram_parameter("x_bf", [128, 2 * NPIX], bf16, isOutput=False)
    wqkv_d = nc.declare_dram_parameter("wqkv", [128, 2 * 768], bf16, isOutput=False)
    bqkv_d = nc.declare_dram_parameter("bqkv_c", [128, 6], f32, isOutput=False)
    wo_d = nc.declare_dram_parameter("wo", [128, 2 * 256], bf16, isOutput=False)
    bo_d = nc.declare_dram_parameter("bo_c", [128, 2], f32, isOutput=False)
    taps_d = nc.declare_dram_parameter("taps", [128, 18 * 128], bf16, isOutput=False)
    lepebv_d = nc.declare_dram_parameter("lepebv_c", [128, 2], f32, isOutput=False)
    ridx_d = nc.declare_dram_parameter("ridx", [1, 64], i32, isOutput=False)
    y_d = nc.declare_dram_parameter("y", [128, 2 * NPIX], f32, isOutput=True)

    with TileContext(nc) as tc:
        with (
            tc.tile_pool(name="consts", bufs=1) as cpool,
            tc.tile_pool(name="pwork", bufs=6) as ppool,
            tc.tile_pool(name="kstage", bufs=16) as kpool,
            tc.tile_pool(name="vstage", bufs=16) as vpool,
            tc.tile_pool(name="ystage", bufs=4) as ypool,
            tc.tile_pool(name="ps_big", bufs=2, space="PSUM") as ps_big,
            tc.tile_pool(name="ps_bank", bufs=2, space="PSUM") as ps_bank,
            tc.tile_pool(name="ps_small", bufs=2, space="PSUM") as ps_small,
        ):
            # ---- persistent SBUF tensors ----
            x_sb = cpool.tile([128, 2 * NPIX], bf16, tag="x")
            wqkv_sb = cpool.tile([128, 2 * 768], bf16, tag="wqkv")
            bqkv_sb = cpool.tile([128, 6], f32, tag="bqkv")
            wo_sb = cpool.tile([128, 2 * 256], bf16, tag="wo")
            bo_sb = cpool.tile([128, 2], f32, tag="bo")
            taps_sb = cpool.tile([128, 18 * 128], bf16, tag="taps")
            lepebv_sb = cpool.tile([128, 2], f32, tag="lepebv")
            ridx_sb = cpool.tile([1, 64], i32, tag="ridx")
            ones32_sb = cpool.tile([128, 32], bf16, tag="ones32")
            qkv_sb = cpool.tile([128, 4 * NPIX], bf16, tag="qkv")
            vtok_sb = cpool.tile([128, 2 * NPIX], bf16, tag="vtok")
            attn_sb = cpool.tile([128, 2 * NPIX], bf16, tag="attn")
            recip_sb = cpool.tile([128, 2 * NPIX], bf16, tag="recip")
            lepe_sb = cpool.tile([128, 2 * NPIX], bf16, tag="lepe")
            # zero-padded V image per channel chunk: col = 1 + r*66 + c, one
            # guard column each side of every row -> conv shifts never wrap
            # and all LePE matmul APs stay flat (1-D free).
            vpad_sb = cpool.tile([128, 2 * 4240], bf16, tag="vpad")
            comb_sb = cpool.tile([128, 2 * NPIX], bf16, tag="comb")

            # input DMAs ordered by first use: the opening qkv matmul needs
            # wqkv + the first x half, so those go first on the queue; the
            # second x half overlaps the first matmuls. Small/late tensors
            # ride the gpsimd queue so they don't delay the x stream.
            nc.sync.dma_start(out=wqkv_sb[:], in_=wqkv_d[:])
            nc.sync.dma_start(out=x_sb[:, :NPIX], in_=x_d[:, :NPIX])
            nc.gpsimd.dma_start(out=bqkv_sb[:], in_=bqkv_d[:])
            nc.sync.dma_start(out=x_sb[:, NPIX:], in_=x_d[:, NPIX:])
            nc.gpsimd.dma_start(out=taps_sb[:], in_=taps_d[:])
            nc.gpsimd.dma_start(out=wo_sb[:], in_=wo_d[:])
            nc.gpsimd.dma_start(out=bo_sb[:], in_=bo_d[:])
            nc.gpsimd.dma_start(out=lepebv_sb[:], in_=lepebv_d[:])
            nc.gpsimd.dma_start(out=ridx_sb[:], in_=ridx_d[:])
            nc.vector.memset(ones32_sb[:], 1.0)
            # zero only the guard columns of the padded V image: col 0 plus the
            # (64,65) pair at the end of each row; interior is fully overwritten
            # by the qkv chunk-4/5 drains.
            for m in range(2):
                nc.vector.memset(vpad_sb[:, m * 4240 : m * 4240 + 1], 0.0)
                guards = vpad_sb[
                    :, m * 4240 + 65 : m * 4240 + 65 + 63 * 66
                ].rearrange("p (r c) -> p r c", c=66)[:, :, :2]
                nc.vector.memset(guards, 0.0)
                nc.vector.memset(vpad_sb[:, m * 4240 + 4223 : (m + 1) * 4240], 0.0)

            # ---- phase 1: qkv channel-major [768, 4096] ----
            # out chunk mt (feature rows mt*128..): psum = sum_k Wqkv_k.T @ x_k
            for mt in range(6):
                for nt in range(8):
                    ps = ps_bank.tile([128, 512], f32, tag="bank")
                    for k in range(2):
                        nc.tensor.matmul(
                            out=ps[:],
                            lhsT=wqkv_sb[:, k * 768 + mt * 128 : k * 768 + (mt + 1) * 128],
                            rhs=x_sb[:, k * NPIX + nt * 512 : k * NPIX + (nt + 1) * 512],
                            start=(k == 0),
                            stop=(k == 1),
                        )
                    if mt < 4:
                        dst = qkv_sb[:, mt * NPIX + nt * 512 : mt * NPIX + (nt + 1) * 512]
                    else:
                        # V image chunk: drain into the zero-padded conv layout
                        # (8 rows of 64 at stride 66, starting col 1)
                        m = mt - 4
                        base = m * 4240 + 1 + nt * 8 * 66
                        dst = vpad_sb[:, base : base + 8 * 66].rearrange(
                            "p (r c) -> p r c", c=66
                        )[:, :, :64]
                    nc.vector.tensor_scalar(
                        out=dst, in0=ps[:] if mt < 4 else ps[:].rearrange(
                            "p (r c) -> p r c", c=64
                        ),
                        scalar1=bqkv_sb[:, mt : mt + 1],
                        scalar2=None, op0=mybir.AluOpType.add,
                    )

            # ---- phase 2: V token-major [128 pix, 32w*256] (no bias; folded later) ----
            for wt in range(P2):
                ps = ps_bank.tile([128, 256], f32, tag="bank")
                for k in range(2):
                    nc.tensor.matmul(
                        out=ps[:],
                        lhsT=x_sb[:, k * NPIX + wt * 128 : k * NPIX + (wt + 1) * 128],
                        rhs=wqkv_sb[:, k * 768 + 512 : k * 768 + 768],
                        start=(k == 0),
                        stop=(k == 1),
                    )
                nc.vector.tensor_copy(
                    out=vtok_sb[:, wt * 256 : (wt + 1) * 256], in_=ps[:]
                )

            # ---- phase 4+5: scores -> exp -> AV + denom, grouped by 4 windows ----
            # Matmul stationary operands cannot take register offsets, so the
            # selected K/V windows are gathered into static staging tiles with
            # dynamic-offset DMAs (indices live in SP registers, freed per
            # group). K staging: [128ch, 2chunks*128pix]; V: [128pix, 256ch].
            for wg in range(0, P2, 4):
                jv = {}
                stage = {}
                for w in range(wg, wg + 4):
                    jv[w] = []
                    for s in range(2):
                        v = nc.values_load(
                            ridx_sb[0:1, 2 * w + s : 2 * w + s + 1],
                            engines=[mybir.EngineType.SP, mybir.EngineType.Pool],
                            min_val=0, max_val=31,
                            skip_runtime_bounds_check=True,
                        )
                        jv[w].append(v)
                        kst = kpool.tile([128, 256], bf16, tag="kst")
                        nc.sync.dma_start(
                            out=kst[:, 0:128],
                            in_=qkv_sb[:, ds(2 * NPIX + v * 128, 128)],
                        )
                        nc.gpsimd.dma_start(
                            out=kst[:, 128:256],
                            in_=qkv_sb[:, ds(3 * NPIX + v * 128, 128)],
                        )
                        vst = vpool.tile([128, 256], bf16, tag="vst")
                        nc.gpsimd.dma_start(
                            out=vst[:], in_=vtok_sb[:, ds(v * 256, 256)]
                        )
                        stage[w, s] = (kst, vst)
                ptiles = {}
                for w in range(wg, wg + 4):
                    ptile = ppool.tile([128, 2048], bf16, tag="P")
                    ptiles[w] = ptile
                    # scores in two 2-bank tiles (row-groups 0-1 / 2-3) so the
                    # next window's scores overlap this window's exp; within a
                    # tile, bank b2 holds the four (s, g) blocks of its row
                    # group at col s*256 + g*128
                    psc_a = ps_big.tile([128, 1024], f32, tag="sc")
                    psc_b = ps_big.tile([128, 1024], f32, tag="sc")
                    pscs = [psc_a, psc_b]
                    for half in range(2):
                        for s in range(2):
                            kst, _ = stage[w, s]
                            for b2 in range(2):
                                b = half * 2 + b2
                                for g in range(2):
                                    off = b2 * 512 + s * 256 + g * 128
                                    nc.tensor.matmul(
                                        out=pscs[half][:, off : off + 128],
                                        lhsT=kst[
                                            b * 32 : b * 32 + 32,
                                            g * 128 : (g + 1) * 128,
                                        ],
                                        rhs=qkv_sb[
                                            b * 32 : b * 32 + 32,
                                            g * NPIX + w * 128 : g * NPIX + (w + 1) * 128,
                                        ],
                                        tile_position=(b * 32, 0),
                                        start=True, stop=True,
                                    )
                        # exp for this half: P cols at s*1024 + (b*2+g)*128
                        esrc = pscs[half][:].rearrange(
                            "p (b2 s g q) -> p b2 s g q", s=2, g=2, q=128
                        )
                        edst = ptile[:].rearrange(
                            "p (s hb b2 g q) -> p hb b2 s g q",
                            s=2, hb=2, b2=2, g=2,
                        )[:, half]
                        nc.scalar.activation(
                            out=edst, in_=esrc, func=AF.Exp, scale=float(SCALE),
                        )
                # AV + denom: one bank per (group, hg) holding 4 windows
                for hg in range(2):
                    psav = ps_small.tile([128, 512], f32, tag="avden")
                    psden = ps_small.tile([128, 512], f32, tag="avden")
                    for w in range(wg, wg + 4):
                        ptile = ptiles[w]
                        wc = (w - wg) * 128
                        for h2 in range(4):
                            h = hg * 4 + h2
                            hh = (h % 4) * 2 + h // 4
                            for s in range(2):
                                _, vst = stage[w, s]
                                rhs = ptile[:, s * 1024 + hh * 128 : s * 1024 + (hh + 1) * 128]
                                nc.tensor.matmul(
                                    out=psav[h2 * 32 : (h2 + 1) * 32, wc : wc + 128],
                                    lhsT=vst[:, h * 32 : (h + 1) * 32],
                                    rhs=rhs,
                                    tile_position=(0, h2 * 32),
                                    start=(s == 0), stop=(s == 1),
                                )
                                nc.tensor.matmul(
                                    out=psden[h2 * 32 : (h2 + 1) * 32, wc : wc + 128],
                                    lhsT=ones32_sb[:, :32],
                                    rhs=rhs,
                                    tile_position=(0, h2 * 32),
                                    start=(s == 0), stop=(s == 1),
                                )
                    nc.scalar.activation(
                        out=attn_sb[:, hg * NPIX + wg * 128 : hg * NPIX + (wg + 4) * 128],
                        in_=psav[:], func=AF.Copy,
                    )
                    with nc.allow_low_precision(reason="bf16 softmax recip"):
                        nc.vector.reciprocal(
                            out=recip_sb[:, hg * NPIX + wg * 128 : hg * NPIX + (wg + 4) * 128],
                            in_=psden[:],
                        )
                for w in range(wg, wg + 4):
                    for s in range(2):
                        for r in jv[w][s].val:
                            nc.free_register(r)

            # ---- phase 6: LePE depthwise 3x3 via diagonal matmuls ----
            # padded image: 7-row chunks (N = 7*66 = 462 <= one PSUM bank);
            # row-boundary truncation trims the flat span, column wrap reads
            # the zero guard columns. Pad columns in PSUM accumulate garbage
            # that the drain simply skips.
            NROW = 7
            for m in range(2):
                for pc in range(10):  # ceil(64/7) chunks of NROW rows
                    cr0, cr1 = pc * NROW, min(pc * NROW + NROW, 64)
                    if cr0 >= 64:
                        break
                    ps = ps_bank.tile([128, 512], f32, tag="bank")
                    active = []
                    for t, (dy, dx) in enumerate(TAPS):
                        r0 = max(cr0, -dy)
                        r1 = min(cr1, 64 - dy)
                        if r0 < r1:
                            active.append((t, dy, dx, r0, r1))
                    for i, (t, dy, dx, r0, r1) in enumerate(active):
                        out_ap = ps[:, (r0 - cr0) * 66 : (r1 - cr0) * 66]
                        base = m * 4240 + 1 + (r0 + dy) * 66 + dx
                        src = vpad_sb[:, base : base + (r1 - r0) * 66]
                        nc.tensor.matmul(
                            out=out_ap,
                            lhsT=taps_sb[:, (t * 2 + m) * 128 : (t * 2 + m + 1) * 128],
                            rhs=src,
                            start=(i == 0), stop=(i == len(active) - 1),
                        )
                    drain_src = ps[:, : (cr1 - cr0) * 66].rearrange(
                        "p (r c) -> p r c", c=66
                    )[:, :, :64]
                    nc.scalar.activation(
                        out=lepe_sb[
                            :, m * NPIX + cr0 * 64 : m * NPIX + cr1 * 64
                        ].rearrange("p (r c) -> p r c", c=64),
                        in_=drain_src, func=AF.Identity,
                        bias=lepebv_sb[:, m : m + 1],
                    )

            # ---- phase 7: combine + Wo + bias + out ----
            for m in range(2):
                for nt in range(4):
                    sl = slice(m * NPIX + nt * 1024, m * NPIX + (nt + 1) * 1024)
                    nc.vector.tensor_mul(
                        out=comb_sb[:, sl], in0=attn_sb[:, sl], in1=recip_sb[:, sl]
                    )
                    nc.vector.tensor_add(
                        out=comb_sb[:, sl], in0=comb_sb[:, sl], in1=lepe_sb[:, sl]
                    )
            for mo in range(2):
                for nt in range(8):
                    ps = ps_bank.tile([128, 512], f32, tag="bank")
                    for k in range(2):
                        nc.tensor.matmul(
                            out=ps[:],
                            lhsT=wo_sb[:, k * 256 + mo * 128 : k * 256 + (mo + 1) * 128],
                            rhs=comb_sb[:, k * NPIX + nt * 512 : k * NPIX + (nt + 1) * 512],
                            start=(k == 0), stop=(k == 1),
                        )
                    ystg = ypool.tile([128, 512], f32, tag="y")
                    nc.vector.tensor_scalar(
                        out=ystg[:], in0=ps[:], scalar1=bo_sb[:, mo : mo + 1],
                        scalar2=None, op0=mybir.AluOpType.add,
                    )
                    nc.sync.dma_start(
                        out=y_d[:, mo * NPIX + nt * 512 : mo * NPIX + (nt + 1) * 512],
                        in_=ystg[:],
                    )
    nc.finalize()
    return nc


def _host_prep(x, Wqkv, bqkv, Wo, bo, lepe_w, lepe_b):
    """Build the 8 per-core input maps."""
    bf = ml_dtypes.bfloat16
    x = np.asarray(x, np.float32)
    Wqkv = np.asarray(Wqkv, np.float32)
    bqkv = np.asarray(bqkv, np.float32)
    Wo = np.asarray(Wo, np.float32)
    bo = np.asarray(bo, np.float32)
    lepe_w = np.asarray(lepe_w, np.float32)
    lepe_b = np.asarray(lepe_b, np.float32)

    # shared weight tensors
    wqkv_t = np.concatenate([Wqkv[:128, :], Wqkv[128:, :]], axis=1).astype(bf)
    bqkv_c = bqkv.reshape(6, 128).T.copy().astype(np.float32)
    wo_t = np.concatenate([Wo[:128, :], Wo[128:, :]], axis=1).astype(bf)
    bo_c = bo.reshape(2, 128).T.copy().astype(np.float32)
    bv = bqkv[512:768]
    lepebv = (lepe_b + bv).reshape(2, 128).T.copy().astype(np.float32)

    tw = lepe_w[:, :, 0, :]  # [3(dy), 3(dx), 256]

    def taps_mat(twb):
        out = np.zeros((128, 18 * 128), np.float32)
        for t, (dy, dx) in enumerate(TAPS):
            for m in range(2):
                blk = np.diag(twb[dy + 1, dx + 1, m * 128 : (m + 1) * 128])
                out[:, (t * 2 + m) * 128 : (t * 2 + m + 1) * 128] = blk
        return out.astype(bf)

    taps_v = taps_mat(tw)
    taps_h = taps_mat(tw.transpose(1, 0, 2))

    Wq, Wk = Wqkv[:, :QK], Wqkv[:, QK : 2 * QK]
    bq, bk = bqkv[:QK], bqkv[QK : 2 * QK]

    in_maps = []
    for core in range(8):
        n, br = core // 2, core % 2
        img = x[n]  # [256, 64, 64]
        if br == 1:
            img = img.transpose(0, 2, 1)
        xc = np.ascontiguousarray(img.reshape(256, NPIX))

        # exact host routing
        xm = xc.reshape(256, P2, W2).mean(axis=2, dtype=np.float32)  # [256, 32]
        q_win = xm.T @ Wq + bq
        k_win = xm.T @ Wk + bk
        logit = (q_win * SCALE) @ k_win.T
        ridx = np.argsort(-logit, axis=1, kind="stable")[:, :2].astype(np.int32)

        x_bf = np.concatenate([xc[:128, :], xc[128:, :]], axis=1).astype(bf)
        in_maps.append({
            "x_bf": x_bf,
            "wqkv": wqkv_t,
            "bqkv_c": bqkv_c,
            "wo": wo_t,
            "bo_c": bo_c,
            "taps": taps_v if br == 0 else taps_h,
            "lepebv_c": lepebv,
            "ridx": ridx.reshape(1, 64),
        })
    return in_maps


def _host_post(results):
    """results: list of 8 dicts with 'y' [128, 8192] fp32 -> [4,256,64,64]."""
    out = np.zeros((N_B, 256, H, W), np.float32)
    for core, res in enumerate(results):
        n, br = core // 2, core % 2
        y = res["y"]
        yc = np.concatenate([y[:, :NPIX], y[:, NPIX:]], axis=0)  # [256, 4096]
        img = yc.reshape(256, 64, 64)
        if br == 1:
            img = img.transpose(0, 2, 1)
        out[n] += img
    return out


def kernel(x, Wqkv, bqkv, Wo, bo, lepe_w, lepe_b):
    from concourse.bass_utils import run_bass_kernel_spmd

    if "nc" not in _CACHE:
        _CACHE["nc"] = _build_program()
    nc = _CACHE["nc"]
    in_maps = _host_prep(x, Wqkv, bqkv, Wo, bo, lepe_w, lepe_b)
    res = run_bass_kernel_spmd(nc, in_maps, core_ids=list(range(8)))
    return _host_post(res.results)



# revision 19
# speedup vs baseline: 1.4425x; 1.4425x over previous
"""Trainium2 Bass kernel for nn_ARAttention (axial-region top-k windowed attention).

Sharding: 8 cores = 4 batches x 2 branches (vertical / horizontal). The
horizontal branch is the vertical branch on the spatially-transposed image
(with a spatially-transposed LePE kernel), so all cores run one SPMD program.

v3 design (vs baseline):
  - AV is token-major: out[q, ch] = P_block.T @ [V | 1], with P (the exp'd
    scores) as the stationary operand and a 33-wide moving operand per head
    whose last column of ones produces the softmax denominator for free.
    This cuts AV+denominator PE rows 131k -> 17k.
  - The [q, ch] attention output is scaled by the reciprocal denominator on
    DVE (free-dim broadcast), then transposed back to channel-major via an
    identity matmul that accumulates directly onto the LePE taps in PSUM.
  - LePE 3x3 depthwise conv is split: some taps as PE diagonal matmuls into
    the per-window PSUM tile, the rest on DVE as tensor_scalar (4x mode) +
    tensor_tensor (2x) chains over a zero-guarded padded image.
  - K is stored window-interleaved so each top-k selection is ONE dynamic
    DMA; V selections index vtok directly via PE-register offsets (no DMA).
  - Wo streams per 4-window group instead of waiting for all windows.
"""

import numpy as np
import ml_dtypes

DIM = 256
QK = 256
HEADS = 8
HD = 32  # head dim
N_B = 4
H = W = 64
P2 = 32  # windows per branch image
W2 = 128  # pixels per window
NPIX = 4096
SCALE = QK ** -0.5

TAPS = [(0, 0), (-1, -1), (-1, 0), (-1, 1), (0, -1), (0, 1), (1, -1), (1, 0), (1, 1)]

# padded V image layout per channel chunk: one 66-col zero guard row on top
# and bottom, rows of 64 payload + 2 guard cols at stride 66. VBASE has two
# spare cols so corner taps (dy=-1, dx=-1) never index below the buffer.
VROW = 66
VPAD = 4360  # 69 (top guard) + 64*66 + bottom guard
VBASE = VROW + 3  # col of row 0, payload col 0

# LePE tap split: PE taps accumulate in the per-window PSUM tile (merged with
# the attention transpose); DVE taps accumulate into lepe_sb (banded).
PE_TAPS_M0 = [0, 1, 2, 3, 4, 5, 6, 7, 8]  # all m=0 taps on PE
PE_TAPS_M1 = [0]
DVE_FULL_M1 = [1, 2, 3]   # full TS(+bias)/TS+TT chain on DVE
HYB_M1 = [4, 5, 6, 7, 8]  # DVE does the 4x multiply, Pool the add
N_BANDS = 4  # lepe bands of 16 rows each

_CACHE = {}


def _build_program(zero_bias=True):
    import concourse.bass as bass
    import concourse.bacc as bacc
    import concourse.mybir as mybir
    from concourse.bass import ds
    from concourse.tile import TileContext

    f32 = mybir.dt.float32
    bf16 = mybir.dt.bfloat16
    i32 = mybir.dt.int32
    AF = mybir.ActivationFunctionType
    ALU = mybir.AluOpType

    nc = bacc.Bacc("TRN2", target_bir_lowering=False, debug=False)

    x_d = nc.declare_dram_parameter("x_bf", [128, 2 * NPIX], bf16, isOutput=False)
    wqkv_d = nc.declare_dram_parameter("wqkv", [128, 2 * 768], bf16, isOutput=False)
    bqkv_d = nc.declare_dram_parameter("bqkv_c", [128, 6], f32, isOutput=False)
    wo_d = nc.declare_dram_parameter("wo", [128, 2 * 256], bf16, isOutput=False)
    bo_d = nc.declare_dram_parameter("bo_c", [128, 2], f32, isOutput=False)
    tapw_d = nc.declare_dram_parameter("tapw_c", [128, 18], f32, isOutput=False)
    lepebv_d = nc.declare_dram_parameter("lepebv_c", [128, 2], f32, isOutput=False)
    ident_d = nc.declare_dram_parameter("ident", [128, 128], bf16, isOutput=False)
    ridx_d = nc.declare_dram_parameter("ridx", [1, 64], i32, isOutput=False)
    y_d = nc.declare_dram_parameter("y", [128, 2 * NPIX], f32, isOutput=True)

    with TileContext(nc) as tc:
        with (
            tc.tile_pool(name="consts", bufs=1) as cpool,
            tc.tile_pool(name="kstage", bufs=6) as kpool,
            tc.tile_pool(name="pwork", bufs=4) as ppool,
            tc.tile_pool(name="ystage", bufs=4) as ypool,
            tc.tile_pool(name="psum", bufs=2, space="PSUM") as pspool,
        ):
            # ---- persistent SBUF tensors ----
            x_sb = cpool.tile([128, 2 * NPIX], bf16, tag="x")
            wqkv_sb = cpool.tile([128, 2 * 768], bf16, tag="wqkv")
            bqkv_sb = cpool.tile([128, 6], f32, tag="bqkv")
            wo_sb = cpool.tile([128, 2 * 256], bf16, tag="wo")
            bo_sb = cpool.tile([128, 2], f32, tag="bo")
            tapw_sb = cpool.tile([128, 18], f32, tag="tapw")
            lepebv_sb = cpool.tile([128, 2], f32, tag="lepebv")
            ident_sb = cpool.tile([128, 128], bf16, tag="ident")
            ridx_sb = cpool.tile([1, 64], i32, tag="ridx")
            q_sb = cpool.tile([128, 2 * NPIX], bf16, tag="q")  # Q channel-major
            kwin_sb = cpool.tile([128, P2 * 256], bf16, tag="kwin")  # K per window
            vtok_sb = cpool.tile([128, P2 * 264], bf16, tag="vtok")  # [V|1] per win
            vpad_sb = cpool.tile([128, 2 * VPAD], bf16, tag="vpad")
            lepe_sb = cpool.tile([128, 2 * VPAD], bf16, tag="lepe")
            ltmp_sb = cpool.tile([128, 6 * 16 * VROW], bf16, tag="ltmp")
            comb_tm = cpool.tile([128, P2 * 256], bf16, tag="combtm")
            comb_sb = cpool.tile([128, 2 * NPIX], bf16, tag="comb")
            rec_sb = cpool.tile([128, P2 * 8], f32, tag="rec")

            # ---- input DMAs ----
            nc.sync.dma_start(out=wqkv_sb[:], in_=wqkv_d[:])
            nc.sync.dma_start(out=x_sb[:, :NPIX], in_=x_d[:, :NPIX])
            nc.gpsimd.dma_start(out=ridx_sb[:], in_=ridx_d[:])
            nc.gpsimd.dma_start(out=bqkv_sb[:], in_=bqkv_d[:])
            nc.sync.dma_start(out=x_sb[:, NPIX:], in_=x_d[:, NPIX:])
            nc.gpsimd.dma_start(out=tapw_sb[:], in_=tapw_d[:])
            nc.gpsimd.dma_start(out=lepebv_sb[:], in_=lepebv_d[:])
            nc.gpsimd.dma_start(out=ident_sb[:], in_=ident_d[:])
            nc.gpsimd.dma_start(out=wo_sb[:], in_=wo_d[:])
            nc.gpsimd.dma_start(out=bo_sb[:], in_=bo_d[:])

            # ---- memsets: vtok ones columns, vpad zero guards ----
            ones_cols = vtok_sb[:].rearrange(
                "p (w h c) -> p w h c", h=HEADS, c=33
            )[:, :, :, 32:33]
            nc.vector.memset(ones_cols, 1.0)
            for m in range(2):
                base = m * VPAD
                nc.vector.memset(vpad_sb[:, base : base + VBASE], 0.0)
                guards = vpad_sb[
                    :, base + VBASE + 64 : base + VBASE + 64 + 63 * VROW
                ].rearrange("p (r c) -> p r c", c=VROW)[:, :, :2]
                nc.vector.memset(guards, 0.0)
                nc.vector.memset(
                    vpad_sb[:, base + VBASE - 2 + 64 * VROW : base + VPAD], 0.0
                )

            # ---- phase 1: K then Q then V (channel-major projections) ----
            # K chunks (wqkv cols 256..512) -> kwin window-interleaved layout
            for nt in range(8):
                for g in range(2):
                    ps = pspool.tile([128, 512], f32, tag="bank", bufs=4)
                    for k in range(2):
                        nc.tensor.matmul(
                            out=ps[:],
                            lhsT=wqkv_sb[:, k * 768 + 256 + g * 128 : k * 768 + 256 + (g + 1) * 128],
                            rhs=x_sb[:, k * NPIX + nt * 512 : k * NPIX + (nt + 1) * 512],
                            start=(k == 0),
                            stop=(k == 1),
                        )
                    dst = kwin_sb[
                        :, nt * 4 * 256 : (nt + 1) * 4 * 256
                    ].rearrange("p (v g x) -> p v g x", g=2, x=128)[:, :, g, :]
                    src = ps[:].rearrange("p (v x) -> p v x", x=128)
                    # Act drains K (idle pre-exp); bias added during copy
                    nc.scalar.activation(
                        out=dst, in_=src, func=AF.Identity,
                        bias=bqkv_sb[:, 2 + g : 3 + g],
                    )

            def q_block(nt):
                for g in range(2):
                    ps = pspool.tile([128, 512], f32, tag="bank", bufs=4)
                    for k in range(2):
                        nc.tensor.matmul(
                            out=ps[:],
                            lhsT=wqkv_sb[:, k * 768 + g * 128 : k * 768 + (g + 1) * 128],
                            rhs=x_sb[:, k * NPIX + nt * 512 : k * NPIX + (nt + 1) * 512],
                            start=(k == 0),
                            stop=(k == 1),
                        )
                    nc.vector.tensor_scalar(
                        out=q_sb[:, g * NPIX + nt * 512 : g * NPIX + (nt + 1) * 512],
                        in0=ps[:], scalar1=bqkv_sb[:, g : g + 1], scalar2=None,
                        op0=ALU.add,
                    )

            def v_block(nt):
                for m in range(2):
                    ps = pspool.tile([128, 512], f32, tag="bank", bufs=4)
                    for k in range(2):
                        nc.tensor.matmul(
                            out=ps[:],
                            lhsT=wqkv_sb[:, k * 768 + 512 + m * 128 : k * 768 + 512 + (m + 1) * 128],
                            rhs=x_sb[:, k * NPIX + nt * 512 : k * NPIX + (nt + 1) * 512],
                            start=(k == 0),
                            stop=(k == 1),
                        )
                    base = m * VPAD + VBASE + nt * 8 * VROW
                    dst = vpad_sb[:, base : base + 8 * VROW].rearrange(
                        "p (r c) -> p r c", c=VROW
                    )[:, :, :64]
                    nc.vector.tensor_scalar(
                        out=dst, in0=ps[:].rearrange("p (r c) -> p r c", c=64),
                        scalar1=bqkv_sb[:, 4 + m : 5 + m], scalar2=None,
                        op0=ALU.add,
                    )

            def p2_block(wt):
                # V token-major [128 pix, 256 ch] for two windows per psum
                # tile -> vtok (no bias; folded in lepe bias)
                ps = pspool.tile([128, 512], f32, tag="bank", bufs=4)
                for j in range(2):
                    for k in range(2):
                        nc.tensor.matmul(
                            out=ps[:, j * 256 : (j + 1) * 256],
                            lhsT=x_sb[:, k * NPIX + (wt + j) * 128 : k * NPIX + (wt + j + 1) * 128],
                            rhs=wqkv_sb[:, k * 768 + 512 : k * 768 + 768],
                            start=(k == 0),
                            stop=(k == 1),
                        )
                dst = vtok_sb[:, wt * 264 : (wt + 2) * 264].rearrange(
                    "p (w h c) -> p w h c", h=HEADS, c=33
                )[:, :, :, :32]
                nc.vector.tensor_copy(
                    out=dst,
                    in_=ps[:].rearrange("p (w h c) -> p w h c", h=HEADS, c=32),
                )

            # ---- per-window machinery ----
            jvals = {}

            def load_idx(w):
                vs = []
                for s in range(2):
                    eng_dma = mybir.EngineType.SP if (w % 2 == 0) else mybir.EngineType.Pool
                    v = nc.values_load(
                        ridx_sb[0:1, 2 * w + s : 2 * w + s + 1],
                        engines=[mybir.EngineType.PE, eng_dma],
                        min_val=0, max_val=31,
                        skip_runtime_bounds_check=True,
                    )
                    vs.append(v)
                jvals[w] = vs

            kst_tiles = {}

            def kst_gather(w):
                tiles = []
                for s in range(2):
                    v = jvals[w][s]
                    kst = kpool.tile([128, 256], bf16, tag="kst")
                    eng = nc.sync if (w % 2 == 0) else nc.gpsimd
                    eng.dma_start(out=kst[:], in_=kwin_sb[:, ds(v * 256, 256)])
                    tiles.append(kst)
                kst_tiles[w] = tiles

            ptiles = {}

            def scores(w):
                ptile = ppool.tile([128, 2048], bf16, tag="P")
                ptiles[w] = ptile
                psc_a = pspool.tile([128, 1024], f32, tag="sc")
                psc_b = pspool.tile([128, 1024], f32, tag="sc")
                pscs = [psc_a, psc_b]
                for half in range(2):
                    for s in range(2):
                        kst = kst_tiles[w][s]
                        for b2 in range(2):
                            b = half * 2 + b2
                            for g in range(2):
                                off = b2 * 512 + s * 256 + g * 128
                                nc.tensor.matmul(
                                    out=pscs[half][:, off : off + 128],
                                    lhsT=kst[
                                        b * 32 : b * 32 + 32,
                                        g * 128 : (g + 1) * 128,
                                    ],
                                    rhs=q_sb[
                                        b * 32 : b * 32 + 32,
                                        g * NPIX + w * 128 : g * NPIX + (w + 1) * 128,
                                    ],
                                    tile_position=(b * 32, 0),
                                    start=True, stop=True,
                                )
                    esrc = pscs[half][:].rearrange(
                        "p (b2 s g q) -> p b2 s g q", s=2, g=2, q=128
                    )
                    edst = ptile[:].rearrange(
                        "p (s hb b2 g q) -> p hb b2 s g q",
                        s=2, hb=2, b2=2, g=2,
                    )[:, half]
                    nc.scalar.activation(
                        out=edst, in_=esrc, func=AF.Exp, scale=float(SCALE),
                    )
                del kst_tiles[w]

            psavs = {}

            def av(w):
                # token-major AV: psAV[q, h*33 + (c|den)] accumulated over s
                psav = pspool.tile([128, 512], f32, tag="bank", bufs=4)
                psavs[w] = psav
                ptile = ptiles[w]
                vsels = [
                    vtok_sb[:, ds(jvals[w][s] * 264, 264)].rearrange(
                        "p (h c) -> p h c", c=33
                    )
                    for s in range(2)
                ]
                for h in range(HEADS):
                    hh = (h % 4) * 2 + h // 4
                    for s in range(2):
                        nc.tensor.matmul(
                            out=psav[:, h * 33 : (h + 1) * 33],
                            lhsT=ptile[:, s * 1024 + hh * 128 : s * 1024 + (hh + 1) * 128],
                            rhs=vsels[s][:, h, :],
                            start=(s == 0), stop=(s == 1),
                        )
                for s in range(2):
                    for r in jvals[w][s].val:
                        nc.free_register(r)
                del jvals[w], ptiles[w]

            def recip_dm(w):
                # reciprocal of the denominators, then scale+drain token-major
                psav = psavs[w]
                dens = psav[:, :264].rearrange("p (h c) -> p h c", c=33)[:, :, 32]
                nc.vector.reciprocal(out=rec_sb[:, w * 8 : (w + 1) * 8], in_=dens)
                rb = rec_sb[:, w * 8 : (w + 1) * 8].unsqueeze(2).broadcast_to(
                    (128, 8, 32)
                )
                with nc.allow_low_precision(reason="bf16 attn scale"):
                    nc.vector.tensor_tensor(
                        out=comb_tm[:, w * 256 : (w + 1) * 256].rearrange(
                            "p (h c) -> p h c", c=32
                        ),
                        in0=psav[:, :264].rearrange("p (h c) -> p h c", c=33)[:, :, :32],
                        in1=rb, op=ALU.mult,
                    )
                del psavs[w]

            def lepe_pe_and_transpose_pair(w0):
                # psum pair tile: col m*256 + j*128 holds (chunk m, window
                # w0+j): PE lepe taps + attn^T accumulate per region
                pst = pspool.tile([128, 512], f32, tag="bank", bufs=4)
                for j in range(2):
                    w = w0 + j
                    for m in range(2):
                        pe_taps = PE_TAPS_M0 if m == 0 else PE_TAPS_M1
                        out = pst[:, m * 256 + j * 128 : m * 256 + (j + 1) * 128]
                        for i, t in enumerate(pe_taps):
                            dy, dx = TAPS[t]
                            base = m * VPAD + VBASE + (2 * w + dy) * VROW + dx
                            src = vpad_sb[:, base : base + 2 * VROW].rearrange(
                                "p (r c) -> p r c", c=VROW
                            )[:, :, :64]
                            nc.tensor.matmul(
                                out=out.rearrange("p (r c) -> p r c", c=64),
                                lhsT=_diag_ap(m, t),
                                rhs=src,
                                start=(i == 0), stop=False,
                            )
                        # transpose attn (scaled, token-major) onto the taps
                        lhsT = comb_tm[
                            :, w * 256 + m * 128 : w * 256 + (m + 1) * 128
                        ]
                        nc.tensor.matmul(
                            out=out,
                            lhsT=lhsT,
                            rhs=ident_sb[:],
                            start=(len(pe_taps) == 0), stop=True,
                        )
                return pst

            def comb_drain_pair(w0, pst):
                # m0: all lepe taps are already in PSUM -> bias-add drain;
                # m1: add the DVE/Pool lepe partial from lepe_sb
                out0 = comb_sb[:, w0 * 128 : (w0 + 2) * 128]
                nc.vector.tensor_scalar(
                    out=out0, in0=pst[:, 0:256],
                    scalar1=lepebv_sb[:, 0:1], scalar2=None, op0=ALU.add,
                )
                lbase = VPAD + VBASE + 2 * w0 * VROW
                lsrc = lepe_sb[:, lbase : lbase + 4 * VROW].rearrange(
                    "p (r c) -> p r c", c=VROW
                )[:, :, :64]
                with nc.allow_low_precision(reason="bf16 comb"):
                    nc.vector.tensor_tensor(
                        out=comb_sb[
                            :, NPIX + w0 * 128 : NPIX + (w0 + 2) * 128
                        ].rearrange("p (r c) -> p r c", c=64),
                        in0=pst[:, 256:512].rearrange("p (r c) -> p r c", c=64),
                        in1=lsrc, op=ALU.add,
                    )

            def _band_aps(m, band, t):
                dy, dx = TAPS[t]
                r0 = band * 16
                span = 16 * VROW
                obase = m * VPAD + VBASE - 1 + r0 * VROW
                ibase = m * VPAD + VBASE - 1 + (r0 + dy) * VROW + dx
                acc = lepe_sb[:, obase : obase + span]
                src = vpad_sb[:, ibase : ibase + span]
                wcol = tapw_sb[:, t * 2 + m : t * 2 + m + 1]
                return acc, src, wcol

            def lepe_dve_band(band):
                # m1 chain head + first taps fully on DVE (tap 1 carries the
                # (lepe_b + bv) bias); hybrid taps: DVE multiplies into ltmp
                # slots, Pool tensor_tensor-adds them onto the accumulator
                for i, t in enumerate(DVE_FULL_M1):
                    acc, src, wcol = _band_aps(1, band, t)
                    if i == 0:
                        nc.vector.tensor_scalar(
                            out=acc, in0=src, scalar1=wcol,
                            scalar2=lepebv_sb[:, 1:2],
                            op0=ALU.mult, op1=ALU.add,
                        )
                    else:
                        tmp = ltmp_sb[:, : 16 * VROW]
                        nc.vector.tensor_scalar(
                            out=tmp, in0=src, scalar1=wcol, scalar2=None,
                            op0=ALU.mult,
                        )
                        with nc.allow_low_precision(reason="bf16 lepe acc"):
                            nc.vector.tensor_tensor(
                                out=acc, in0=tmp, in1=acc, op=ALU.add,
                            )
                for j, t in enumerate(HYB_M1):
                    _, src, wcol = _band_aps(1, band, t)
                    tmp = ltmp_sb[:, (j + 1) * 16 * VROW : (j + 2) * 16 * VROW]
                    nc.vector.tensor_scalar(
                        out=tmp, in0=src, scalar1=wcol, scalar2=None,
                        op0=ALU.mult,
                    )

            pool_lepe_pending = []

            def lepe_pool_band(band):
                # queue the m1 hybrid adds as Pool tensor_tensor thunks
                for j, t in enumerate(HYB_M1):
                    def op(t=t, j=j, band=band):
                        acc, _, _ = _band_aps(1, band, t)
                        tmp = ltmp_sb[:, (j + 1) * 16 * VROW : (j + 2) * 16 * VROW]
                        with nc.allow_low_precision(reason="bf16 lepe acc"):
                            nc.gpsimd.tensor_tensor(
                                out=acc, in0=tmp, in1=acc, op=ALU.add,
                            )
                    pool_lepe_pending.append(op)

            def drain_pool_lepe(n):
                for _ in range(min(n, len(pool_lepe_pending))):
                    pool_lepe_pending.pop(0)()

            # diagonal tap matrices for the PE taps, built once in SBUF from
            # tapw via iota-predicated writes would cost engine time; instead
            # host ships them inside ident_d? No: build with memset+copy is
            # wasteful. They are constructed on host into tapd_d below.
            # (placeholder replaced after pools: see tapd_sb)
            tapd_needed = sorted(
                {(m, t) for m, taps in ((0, PE_TAPS_M0), (1, PE_TAPS_M1))
                 for t in taps}
            )
            # fallthrough: tapd_sb defined before use via closure
            def _diag_ap(m, t):
                i = tapd_needed.index((m, t))
                return tapd_sb[:, i * 128 : (i + 1) * 128]

            def wo_group(nt):
                for mo in range(2):
                    ps = pspool.tile([128, 1024], f32, tag="sc")
                    for k in range(2):
                        nc.tensor.matmul(
                            out=ps[:, :512],
                            lhsT=wo_sb[:, k * 256 + mo * 128 : k * 256 + (mo + 1) * 128],
                            rhs=comb_sb[:, k * NPIX + nt * 512 : k * NPIX + (nt + 1) * 512],
                            start=(k == 0), stop=(k == 1),
                        )
                    ydst = y_d[:, mo * NPIX + nt * 512 : mo * NPIX + (nt + 1) * 512]
                    ystg = ypool.tile([128, 512], f32, tag="y")
                    nc.vector.tensor_scalar(
                        out=ystg[:], in0=ps[:, :512],
                        scalar1=bo_sb[:, mo : mo + 1],
                        scalar2=None, op0=ALU.add,
                    )
                    nc.sync.dma_start(out=ydst, in_=ystg[:])

            # ---- tap diagonal matrices (host-shipped) ----
            ntapd = len(tapd_needed)
            tapd_d = nc.declare_dram_parameter(
                "tapd", [128, max(ntapd, 1) * 128], bf16, isOutput=False
            )
            tapd_sb = cpool.tile([128, max(ntapd, 1) * 128], bf16, tag="tapd")
            nc.gpsimd.dma_start(out=tapd_sb[:], in_=tapd_d[:])

            # ================= schedule =================
            # phase 1 K already emitted above (first in PE order).
            for w in range(4):
                load_idx(w)
                kst_gather(w)
            q_block(0)
            scores(0)
            scores(1)
            q_block(1)
            load_idx(4); kst_gather(4)
            scores(2)
            q_block(2)
            load_idx(5); kst_gather(5)
            scores(3)
            # remaining Q blocks interleaved with scores 4..8
            for i, nt in enumerate(range(3, 8)):
                q_block(nt)
                w = 4 + i
                load_idx(w + 2); kst_gather(w + 2)
                scores(w)
            # V blocks interleaved with scores 9..16
            for nt in range(8):
                v_block(nt)
                w = 9 + nt
                if w <= 31:
                    load_idx(w + 2); kst_gather(w + 2)
                    scores(w)
            # phase 2 (vtok, paired) interleaved with scores 17..24
            for gblk in range(8):
                p2_block(gblk * 4)
                p2_block(gblk * 4 + 2)
                w = 17 + gblk
                load_idx(w + 2); kst_gather(w + 2)
                scores(w)
            # lepe band 0 right after vpad complete
            lepe_dve_band(0)
            lepe_pool_band(0)
            drain_pool_lepe(7)

            # ---- main window stream (pairs) ----
            for wp in range(P2 // 2):
                w0 = 2 * wp
                for ws in (25 + 2 * wp, 26 + 2 * wp):
                    if ws <= 31:
                        if ws >= 27:
                            load_idx(ws); kst_gather(ws)
                        scores(ws)
                drain_pool_lepe(3)
                av(w0)
                recip_dm(w0)
                av(w0 + 1)
                recip_dm(w0 + 1)
                pst = lepe_pe_and_transpose_pair(w0)
                comb_drain_pair(w0, pst)
                if wp in (1, 4, 7):
                    band = (wp + 2) // 3
                    lepe_dve_band(band)
                    lepe_pool_band(band)
                if wp % 2 == 1:
                    wo_group(wp // 2)
            drain_pool_lepe(100)

    nc.finalize()
    return nc


def _host_prep(x, Wqkv, bqkv, Wo, bo, lepe_w, lepe_b):
    """Build the 8 per-core input maps."""
    bf = ml_dtypes.bfloat16
    x = np.asarray(x, np.float32)
    Wqkv = np.asarray(Wqkv, np.float32)
    bqkv = np.asarray(bqkv, np.float32)
    Wo = np.asarray(Wo, np.float32)
    bo = np.asarray(bo, np.float32)
    lepe_w = np.asarray(lepe_w, np.float32)
    lepe_b = np.asarray(lepe_b, np.float32)

    wqkv_t = np.concatenate([Wqkv[:128, :], Wqkv[128:, :]], axis=1).astype(bf)
    bqkv_c = bqkv.reshape(6, 128).T.copy().astype(np.float32)
    wo_t = np.concatenate([Wo[:128, :], Wo[128:, :]], axis=1).astype(bf)
    bo_c = bo.reshape(2, 128).T.copy().astype(np.float32)
    bv = bqkv[512:768]
    lepebv = (lepe_b + bv).reshape(2, 128).T.copy().astype(np.float32)
    ident = np.eye(128, dtype=np.float32).astype(bf)

    tw = lepe_w[:, :, 0, :]  # [3(dy), 3(dx), 256]

    def tapw_mat(twb):
        out = np.zeros((128, 18), np.float32)
        for t, (dy, dx) in enumerate(TAPS):
            for m in range(2):
                out[:, t * 2 + m] = twb[dy + 1, dx + 1, m * 128 : (m + 1) * 128]
        return out

    def tapd_mat(twb):
        need = sorted({(m, t) for m, taps in ((0, PE_TAPS_M0), (1, PE_TAPS_M1))
                       for t in taps})
        out = np.zeros((128, max(len(need), 1) * 128), np.float32)
        for i, (m, t) in enumerate(need):
            dy, dx = TAPS[t]
            out[:, i * 128 : (i + 1) * 128] = np.diag(
                twb[dy + 1, dx + 1, m * 128 : (m + 1) * 128]
            )
        return out.astype(bf)

    tapw_v, tapd_v = tapw_mat(tw), tapd_mat(tw)
    twh = tw.transpose(1, 0, 2)
    tapw_h, tapd_h = tapw_mat(twh), tapd_mat(twh)

    Wq, Wk = Wqkv[:, :QK], Wqkv[:, QK : 2 * QK]
    bq, bk = bqkv[:QK], bqkv[QK : 2 * QK]

    in_maps = []
    for core in range(8):
        n, br = core // 2, core % 2
        img = x[n]  # [256, 64, 64]
        if br == 1:
            img = img.transpose(0, 2, 1)
        xc = np.ascontiguousarray(img.reshape(256, NPIX))

        # exact host routing
        xm = xc.reshape(256, P2, W2).mean(axis=2, dtype=np.float32)
        q_win = xm.T @ Wq + bq
        k_win = xm.T @ Wk + bk
        logit = (q_win * SCALE) @ k_win.T
        ridx = np.argsort(-logit, axis=1, kind="stable")[:, :2].astype(np.int32)

        x_bf = np.concatenate([xc[:128, :], xc[128:, :]], axis=1).astype(bf)
        in_maps.append({
            "x_bf": x_bf,
            "wqkv": wqkv_t,
            "bqkv_c": bqkv_c,
            "wo": wo_t,
            "bo_c": bo_c,
            "tapw_c": tapw_v if br == 0 else tapw_h,
            "tapd": tapd_v if br == 0 else tapd_h,
            "lepebv_c": lepebv,
            "ident": ident,
            "ridx": ridx.reshape(1, 64),
        })
    return in_maps


def _host_post(results):
    """results: list of 8 dicts with 'y' [128, 8192] fp32 -> [4,256,64,64]."""
    out = np.zeros((N_B, 256, H, W), np.float32)
    for core, res in enumerate(results):
        n, br = core // 2, core % 2
        y = res["y"]
        yc = np.concatenate([y[:, :NPIX], y[:, NPIX:]], axis=0)
        img = yc.reshape(256, 64, 64)
        if br == 1:
            img = img.transpose(0, 2, 1)
        out[n] += img
    return out


def kernel(x, Wqkv, bqkv, Wo, bo, lepe_w, lepe_b):
    from concourse.bass_utils import run_bass_kernel_spmd

    zb = (not np.asarray(bqkv, np.float32).any()) and (
        not np.asarray(bo, np.float32).any()
    )
    key = ("nc", bool(zb))
    if key not in _CACHE:
        _CACHE[key] = _build_program(zero_bias=zb)
    nc = _CACHE[key]
    in_maps = _host_prep(x, Wqkv, bqkv, Wo, bo, lepe_w, lepe_b)
    res = run_bass_kernel_spmd(nc, in_maps, core_ids=list(range(8)))
    return _host_post(res.results)
